# revision 7
# baseline (speedup 1.0000x reference)
"""GQA attention kernel for Trainium2, tensor-parallel across 8 NeuronCores.

Problem: B=2, T=2048, D=2048, H=32 q-heads, G=8 kv-heads (GQA, rep=4), hd=64,
causal softmax attention + output projection, fp32 I/O.

Sharding (one KV group per core), with on-device collectives and fp16 I/O to
minimize host<->device traffic:
  Upload per core c (all fp16): x[:, :, c*256:(c+1)*256] (its 256 dims,
  untransposed), Wq[:, c*256:(c+1)*256], Wk/Wv[:, c*64:(c+1)*64],
  Wo[c*256:(c+1)*256, :].
  On device: XBAR DMA-transpose the x slice, AllGather over the 8 cores to
  reconstruct the full xT = [D, T] per batch; each core computes its 4 heads'
  attention + partial output projection into a DRAM buffer; ReduceScatter
  sums the 8 row-parallel Wo partials and leaves each core with a disjoint
  1/8 chunk of the output rows (fp16), the only data downloaded.

On-device dataflow per core (all matmuls contract over the partition dim,
fp16 operands, fp32 PSUM accumulation):
  QT = wq.T @ xT        [256, T]  (1/8 scale folded in on PSUM drain)
  [KT; VT] = wkv.T @ xT [128, T]
  KT duplicated to partitions 64..127 so two heads' score matmuls run
  concurrently in disjoint PE row groups (contraction = hd = 64).
  ST_r = K @ QT_r       [128k, 512q] blocks, causal block-skipped
  PT_r = exp(ST_r)      (ACT, fp16 out; diag blocks masked via affine_select)
  OT_r = [V|1].T @ PT_r [65, 512] (fp16, psum-accumulated over k tiles;
                         row 64 = softmax denominators)
  OT normalized by 1/denominator (DVE), stored fp16 as Wo lhsT layout.
  partial = OT.T @ wo   [t, 2048] -> DRAM fp16, ReduceScatter -> out rows.
"""

import os
import sys

import numpy as np

for _p in ("/opt/trn_rl_repo", "/root/.axon_site/_ro/trn_rl_repo"):
    if os.path.isdir(_p) and _p not in sys.path:
        sys.path.insert(0, _p)

import concourse.bass as bass  # noqa: E402
import concourse.mybir as mybir  # noqa: E402
import concourse.tile as tile  # noqa: E402
from concourse import bacc  # noqa: E402
from concourse.bass_utils import run_bass_kernel_spmd  # noqa: E402
from concourse.masks import make_identity  # noqa: E402
from contextlib import ExitStack  # noqa: E402

B, T, D = 2, 2048, 2048
G, REP, HD = 8, 4, 64
DQ = REP * HD  # 256 q-dims per core
NCORES = 8
P = 128
TB = 512  # q/t block size
KO = D // P  # 16 contraction subtiles for projections
NT = T // TB  # 4 t-blocks
NKT = T // P  # 16 kpos tiles
TS = B * T // NCORES  # 512 flat output rows owned per core
F32 = mybir.dt.float32
F16 = mybir.dt.float16
AF = mybir.ActivationFunctionType
SCALE = 1.0 / 8.0  # 1/sqrt(HD)


def build_kernel(ctx, tc):
    nc = tc.nc
    xs = nc.dram_tensor("xs", [B, T, DQ], F16, kind="ExternalInput").ap()
    wq = nc.dram_tensor("wq", [D, DQ], F16, kind="ExternalInput").ap()
    wkv = nc.dram_tensor("wkv", [D, 2 * HD], F16, kind="ExternalInput").ap()
    wo = nc.dram_tensor("wo", [DQ, D], F16, kind="ExternalInput").ap()
    out = nc.dram_tensor("out", [B, TS // B, D], F16, kind="ExternalOutput").ap()

    dram = ctx.enter_context(tc.tile_pool(name="dram", bufs=1, space="DRAM"))
    xt_loc = dram.tile([P, B, 2, T], F16, tag="xtloc")
    xt_full = dram.tile([NCORES, P, B, 2, T], F16, tag="xtfull", addr_space="Shared")
    partial = dram.tile([B, T, D], F16, tag="partial")
    rs_out = dram.tile([B, TS // B, D], F16, tag="rsout")

    wpool = ctx.enter_context(tc.tile_pool(name="w", bufs=1))
    xt1_pool = ctx.enter_context(tc.tile_pool(name="xt1", bufs=1))
    qt_pool = ctx.enter_context(tc.tile_pool(name="qt", bufs=2))
    kkt_pool = ctx.enter_context(tc.tile_pool(name="kkt", bufs=2))
    vt_pool = ctx.enter_context(tc.tile_pool(name="vt", bufs=2))
    v_pool = ctx.enter_context(tc.tile_pool(name="v", bufs=2))
    xt_pool = ctx.enter_context(tc.tile_pool(name="xt", bufs=6))
    p_pool = ctx.enter_context(tc.tile_pool(name="p", bufs=3))
    o_pool = ctx.enter_context(tc.tile_pool(name="ot", bufs=2))
    r_pool = ctx.enter_context(tc.tile_pool(name="rcp", bufs=3))
    pp = ctx.enter_context(tc.tile_pool(name="pp", bufs=2, space="PSUM"))
    sp = pp
    op = pp
    wp = pp

    # persistent weights (fp16)
    wq_sb = wpool.tile([P, KO, DQ], F16, tag="wq")
    nc.gpsimd.dma_start(wq_sb[:], wq.rearrange("(ko p) m -> p ko m", p=P))
    wkv_sb = wpool.tile([P, KO, 2 * HD], F16, tag="wkv")
    nc.gpsimd.dma_start(wkv_sb[:], wkv.rearrange("(ko p) m -> p ko m", p=P))
    wo_sb = wpool.tile([P, DQ // P, D], F16, tag="wo")
    nc.gpsimd.dma_start(wo_sb[:], wo.rearrange("(ko p) m -> p ko m", p=P))
    ident = wpool.tile([P, P], F32, tag="ident")
    make_identity(nc, ident[:])

    # ------- phase 0: XBAR DMA-transpose local x slice + AllGather ----------
    xt_sb = xt1_pool.tile([P, B, 2, T], F16, tag="xtsb")
    for b in range(B):
        for h in range(2):
            nc.sync.dma_start(
                out=xt_sb[:, b, h, :],
                in_=xs[b, :, h * P : (h + 1) * P],
                transpose=True,
            )
    nc.sync.dma_start(xt_loc[:, :, :, :], xt_sb[:])
    nc.gpsimd.collective_compute(
        "AllGather",
        mybir.AluOpType.bypass,
        replica_groups=[list(range(NCORES))],
        ins=[xt_loc.opt()],
        outs=[xt_full.opt()],
    )

    for b in range(B):
        # ---------------- projections ----------------
        qt_sb = qt_pool.tile([P, 2, T], F16, tag="qt")  # QT, scaled by 1/8
        kkt_sb = kkt_pool.tile([P, T], F16, tag="kkt")  # KT duplicated twice
        vt_sb = vt_pool.tile([P, T], F32, tag="vt")  # VT on partitions 64..127
        for tb in range(NT):
            q_ps0 = pp.tile([P, TB], F32, tag="A")
            q_ps1 = pp.tile([P, TB], F32, tag="B")
            kv_ps = pp.tile([P, TB], F32, tag="C")
            for ko in range(KO):
                xt = xt_pool.tile([P, TB], F16, tag="xt")
                nc.gpsimd.dma_start(
                    xt[:],
                    xt_full[ko // 2, :, b, ko % 2, tb * TB : (tb + 1) * TB],
                )
                st, sp_ = (ko == 0), (ko == KO - 1)
                nc.tensor.matmul(
                    q_ps0[:], wq_sb[:, ko, 0:P], xt[:], start=st, stop=sp_
                )
                nc.tensor.matmul(
                    q_ps1[:], wq_sb[:, ko, P:DQ], xt[:], start=st, stop=sp_
                )
                nc.tensor.matmul(
                    kv_ps[:], wkv_sb[:, ko, :], xt[:], start=st, stop=sp_
                )
            ts = slice(tb * TB, (tb + 1) * TB)
            nc.scalar.activation(qt_sb[:, 0, ts], q_ps0[:], AF.Copy, scale=SCALE)
            nc.scalar.activation(qt_sb[:, 1, ts], q_ps1[:], AF.Copy, scale=SCALE)
            nc.vector.tensor_copy(kkt_sb[0:HD, ts], kv_ps[0:HD, :])
            nc.vector.tensor_copy(vt_sb[HD:P, ts], kv_ps[HD:P, :])
            # duplicate KT to partitions 64..127 (SBUF->SBUF DMA moves partitions)
            nc.sync.dma_start(kkt_sb[HD:P, ts], kkt_sb[0:HD, ts])

        # ---------------- V transpose -> [kpos, hd|1] fp16 ----------------
        v1_sb = v_pool.tile([P, NKT, HD + 1], F16, tag="v1")
        nc.gpsimd.memset(v1_sb[:, :, HD : HD + 1], 1.0)
        for kt in range(NKT):
            tr_ps = wp.tile([P, TB], F32, tag="D")
            nc.tensor.transpose(
                tr_ps[:, 0:HD],
                vt_sb[HD:P, kt * P : (kt + 1) * P],
                ident[HD:P, HD:P],
            )
            nc.vector.tensor_copy(v1_sb[:, kt, 0:HD], tr_ps[:, 0:HD])

        # ---------------- attention + output proj, per q-block ----------------
        for qb in range(NT):
            qs = slice(qb * TB, (qb + 1) * TB)
            nkt = 4 * (qb + 1)  # causal: kpos tiles 0..nkt-1
            ot_sb = o_pool.tile([P, 2, TB], F16, tag="ot")
            for pair in range(2):
                o_ps = []
                for i in range(2):
                    o_ps_i = op.tile([P, TB], F32, tag="C", name=f"o_ps_{i}")
                    o_ps.append(o_ps_i)
                for kt in range(nkt):
                    ks = slice(kt * P, (kt + 1) * P)
                    s_ps0 = sp.tile([P, TB], F32, tag="A")
                    s_ps1 = sp.tile([P, TB], F32, tag="B")
                    nc.tensor.matmul(
                        s_ps0[:],
                        kkt_sb[0:HD, ks],
                        qt_sb[0:HD, pair, qs],
                        start=True,
                        stop=True,
                        tile_position=(0, 0),
                    )
                    nc.tensor.matmul(
                        s_ps1[:],
                        kkt_sb[HD:P, ks],
                        qt_sb[HD:P, pair, qs],
                        start=True,
                        stop=True,
                        tile_position=(64, 0),
                    )
                    pt0 = p_pool.tile([P, TB], F16, tag="p0")
                    pt1 = p_pool.tile([P, TB], F16, tag="p1")
                    nc.scalar.activation(pt0[:], s_ps0[:], AF.Exp)
                    nc.scalar.activation(pt1[:], s_ps1[:], AF.Exp)
                    if kt >= qb * 4:  # diagonal block: causal mask
                        for pt in (pt0, pt1):
                            nc.gpsimd.affine_select(
                                out=pt[:],
                                in_=pt[:],
                                compare_op=mybir.AluOpType.is_ge,
                                fill=0.0,
                                base=qb * TB - kt * P,
                                channel_multiplier=-1,
                                pattern=[[1, TB]],
                            )
                    st, sp_ = (kt == 0), (kt == nkt - 1)
                    nc.tensor.matmul(
                        o_ps[0][0 : HD + 1, :], v1_sb[:, kt, :], pt0[:],
                        start=st, stop=sp_,
                    )
                    nc.tensor.matmul(
                        o_ps[1][0 : HD + 1, :], v1_sb[:, kt, :], pt1[:],
                        start=st, stop=sp_,
                    )
                # normalize: ot[r] = o_ps[r][:64] / o_ps[r][64]
                for i in range(2):
                    sums = r_pool.tile([1, TB], F32, tag="sums")
                    nc.vector.tensor_copy(sums[:], o_ps[i][HD : HD + 1, :])
                    rb = r_pool.tile([HD, TB], F32, tag="rb")
                    nc.gpsimd.partition_broadcast(rb[:], sums[:])
                    nc.vector.reciprocal(rb[:], rb[:])
                    nc.vector.tensor_mul(
                        ot_sb[i * HD : (i + 1) * HD, pair, :],
                        o_ps[i][0:HD, :],
                        rb[:],
                    )
            # Wo partial for this q-block's 512 tokens
            for tt in range(4):
                rows = slice(qb * TB + tt * P, qb * TB + (tt + 1) * P)
                lslice = slice(tt * P, (tt + 1) * P)
                for nb in range(4):
                    wo_ps = wp.tile([P, TB], F32, tag="D")
                    for ko in range(2):
                        nc.tensor.matmul(
                            wo_ps[:],
                            ot_sb[:, ko, lslice],
                            wo_sb[:, ko, nb * TB : (nb + 1) * TB],
                            start=(ko == 0),
                            stop=(ko == 1),
                        )
                    stg = p_pool.tile([P, TB], F16, tag="stg")
                    nc.vector.tensor_copy(stg[:], wo_ps[:])
                    nc.sync.dma_start(
                        partial[b, rows, nb * TB : (nb + 1) * TB], stg[:]
                    )

        # ReduceScatter this batch's row-parallel Wo partials; batch 0's
        # collective overlaps batch 1's compute.
        nc.gpsimd.collective_compute(
            "ReduceScatter",
            mybir.AluOpType.add,
            replica_groups=[list(range(NCORES))],
            ins=[partial[b, :, :].opt()],
            outs=[rs_out[b, :, :].opt()],
        )
    nc.sync.dma_start(out, rs_out[:, :, :])


_NC_CACHE = {}


def get_nc():
    if "nc" not in _NC_CACHE:
        nc = bacc.Bacc("TRN2", target_bir_lowering=False, debug=False)
        with tile.TileContext(nc) as tc, ExitStack() as ctx:
            build_kernel(ctx, tc)
        nc.compile()
        _NC_CACHE["nc"] = nc
    return _NC_CACHE["nc"]


def _f16(a):
    a = np.asarray(a)
    return a if a.dtype == np.float16 else a.astype(np.float16)


def make_in_maps(x, Wq, Wk, Wv, Wo):
    # Convert each tensor to fp16 once (contiguous, vectorized); per-core
    # entries are views — run_bass_kernel_spmd's concat does the only copy.
    x, Wq, Wo = _f16(x), _f16(Wq), _f16(Wo)
    # [D, G, 2*HD] with K in cols :HD and V in HD:, so kv[:, g] is core g's wkv
    kv = np.concatenate(
        [_f16(Wk).reshape(D, G, HD), _f16(Wv).reshape(D, G, HD)], axis=2
    )
    in_maps = []
    for g in range(NCORES):
        dsl = slice(g * DQ, (g + 1) * DQ)
        in_maps.append(
            {"xs": x[:, :, dsl], "wq": Wq[:, dsl], "wkv": kv[:, g], "wo": Wo[dsl, :]}
        )
    return in_maps


def run(x, Wq, Wk, Wv, Wo, trace=False):
    nc = get_nc()
    in_maps = make_in_maps(x, Wq, Wk, Wv, Wo)
    res = run_bass_kernel_spmd(nc, in_maps, list(range(NCORES)), trace=trace)
    full = np.empty((B, T, D), np.float32)
    for c, r in enumerate(res.results):
        full[:, c * (TS // B) : (c + 1) * (TS // B), :] = r["out"]
    return full, res


def kernel(x, Wq, Wk, Wv, Wo):
    return run(x, Wq, Wk, Wv, Wo)[0]


# revision 14
# speedup vs baseline: 1.9214x; 1.9214x over previous
"""GQA attention kernel for Trainium2, tensor-parallel across 8 NeuronCores.

Problem: B=2, T=2048, D=2048, H=32 q-heads, G=8 kv-heads (GQA, rep=4), hd=64,
causal softmax attention + output projection, fp32 I/O.

Sharding (one KV group per core), with on-device collectives and fp16 I/O to
minimize host<->device traffic:
  Upload per core c (all fp16): x[:, :, c*256:(c+1)*256] (its 256 dims,
  untransposed), Wq[:, c*256:(c+1)*256], Wk/Wv[:, c*64:(c+1)*64],
  Wo[c*256:(c+1)*256, :].
  On device: XBAR DMA-transpose the x slice, AllGather over the 8 cores to
  reconstruct the full xT = [D, T] per batch; each core computes its 4 heads'
  attention + partial output projection into a DRAM buffer; ReduceScatter
  sums the 8 row-parallel Wo partials and leaves each core with a disjoint
  1/8 chunk of the output rows (fp16), the only data downloaded.

On-device dataflow per core (all matmuls contract over the partition dim,
fp16 operands, fp32 PSUM accumulation):
  QT = wq.T @ xT        [256, T]  (1/8 scale folded in on PSUM drain)
  [KT; VT] = wkv.T @ xT [128, T]
  KT duplicated to partitions 64..127 so two heads' score matmuls run
  concurrently in disjoint PE row groups (contraction = hd = 64).
  ST_r = K @ QT_r       [128k, 512q] blocks, causal block-skipped
  PT_r = exp(ST_r)      (ACT, fp16 out; diag blocks masked via affine_select)
  OT_r = [V|1].T @ PT_r [65, 512] (fp16, psum-accumulated over k tiles;
                         row 64 = softmax denominators)
  OT normalized by 1/denominator (DVE), stored fp16 as Wo lhsT layout.
  partial = OT.T @ wo   [t, 2048] -> DRAM fp16, ReduceScatter -> out rows.
"""

import os
import sys

import numpy as np

for _p in ("/opt/trn_rl_repo", "/root/.axon_site/_ro/trn_rl_repo"):
    if os.path.isdir(_p) and _p not in sys.path:
        sys.path.insert(0, _p)

import concourse.bass as bass  # noqa: E402
import concourse.mybir as mybir  # noqa: E402
import concourse.tile as tile  # noqa: E402
from concourse import bacc  # noqa: E402
from concourse.bass_utils import run_bass_kernel_spmd  # noqa: E402
from concourse.masks import make_identity  # noqa: E402
from contextlib import ExitStack  # noqa: E402

B, T, D = 2, 2048, 2048
G, REP, HD = 8, 4, 64
DQ = REP * HD  # 256 q-dims per core
NCORES = 8
P = 128
TB = 512  # q/t block size
KO = D // P  # 16 contraction subtiles for projections
NT = T // TB  # 4 t-blocks
NKT = T // P  # 16 kpos tiles
TS = B * T // NCORES  # 512 flat output rows owned per core
F32 = mybir.dt.float32
F16 = mybir.dt.float16
AF = mybir.ActivationFunctionType
SCALE = 1.0 / 8.0  # 1/sqrt(HD)


def build_kernel(ctx, tc):
    nc = tc.nc
    xs = nc.dram_tensor("xs", [B, T, DQ], F16, kind="ExternalInput").ap()
    wq = nc.dram_tensor("wq", [D, DQ], F16, kind="ExternalInput").ap()
    wkv = nc.dram_tensor("wkv", [D, 2 * HD], F16, kind="ExternalInput").ap()
    wo = nc.dram_tensor("wo", [DQ, D], F16, kind="ExternalInput").ap()
    out = nc.dram_tensor("out", [TS, D], F16, kind="ExternalOutput").ap()

    dram = ctx.enter_context(tc.tile_pool(name="dram", bufs=1, space="DRAM"))
    xt_loc = dram.tile([P, B, 2, T], F16, tag="xtloc")
    xt_full = dram.tile([NCORES, P, B, 2, T], F16, tag="xtfull", addr_space="Shared")
    partial = dram.tile([B, T, D], F16, tag="partial")
    rs_out = dram.tile([TS, D], F16, tag="rsout")

    wpool = ctx.enter_context(tc.tile_pool(name="w", bufs=1))
    xt1_pool = ctx.enter_context(tc.tile_pool(name="xt1", bufs=1))
    qt_pool = ctx.enter_context(tc.tile_pool(name="qt", bufs=2))
    kkt_pool = ctx.enter_context(tc.tile_pool(name="kkt", bufs=2))
    vt_pool = ctx.enter_context(tc.tile_pool(name="vt", bufs=2))
    v_pool = ctx.enter_context(tc.tile_pool(name="v", bufs=2))
    xt_pool = ctx.enter_context(tc.tile_pool(name="xt", bufs=2))
    p_pool = ctx.enter_context(tc.tile_pool(name="p", bufs=3))
    o_pool = ctx.enter_context(tc.tile_pool(name="ot", bufs=2))
    r_pool = ctx.enter_context(tc.tile_pool(name="rcp", bufs=3))
    pp = ctx.enter_context(tc.tile_pool(name="pp", bufs=2, space="PSUM"))
    sp = pp
    op = pp
    wp = pp

    # persistent weights (fp16)
    wq_sb = wpool.tile([P, KO, DQ], F16, tag="wq")
    nc.gpsimd.dma_start(wq_sb[:], wq.rearrange("(ko p) m -> p ko m", p=P))
    wkv_sb = wpool.tile([P, KO, 2 * HD], F16, tag="wkv")
    nc.gpsimd.dma_start(wkv_sb[:], wkv.rearrange("(ko p) m -> p ko m", p=P))
    wo_sb = wpool.tile([P, DQ // P, D], F16, tag="wo")
    nc.gpsimd.dma_start(wo_sb[:], wo.rearrange("(ko p) m -> p ko m", p=P))
    ident = wpool.tile([P, P], F32, tag="ident")
    make_identity(nc, ident[:])

    # ------- phase 0: XBAR DMA-transpose local x slice + AllGather ----------
    xt_sb = xt1_pool.tile([P, B, 2, T], F16, tag="xtsb")
    for b in range(B):
        for h in range(2):
            nc.sync.dma_start(
                out=xt_sb[:, b, h, :],
                in_=xs[b, :, h * P : (h + 1) * P],
                transpose=True,
            )
    nc.sync.dma_start(xt_loc[:, :, :, :], xt_sb[:])
    nc.gpsimd.collective_compute(
        "AllGather",
        mybir.AluOpType.bypass,
        replica_groups=[list(range(NCORES))],
        ins=[xt_loc.opt()],
        outs=[xt_full.opt()],
    )

    for b in range(B):
        # ---------------- projections ----------------
        qt_sb = qt_pool.tile([P, 2, T], F16, tag="qt")  # QT, scaled by 1/8
        kkt_sb = kkt_pool.tile([P, T], F16, tag="kkt")  # KT duplicated twice
        vt_sb = vt_pool.tile([P, T], F32, tag="vt")  # VT on partitions 64..127
        for tb in range(NT):
            q_ps0 = pp.tile([P, TB], F32, tag="A")
            q_ps1 = pp.tile([P, TB], F32, tag="B")
            kv_ps = pp.tile([P, TB], F32, tag="C")
            # two batched DMAs bring all 16 contraction blocks for this t-block
            xt = xt_pool.tile([P, NCORES, 2, TB], F16, tag="xt")
            for h in range(2):
                nc.gpsimd.dma_start(
                    xt[:, :, h, :],
                    xt_full[:, :, b, h, tb * TB : (tb + 1) * TB].rearrange(
                        "c p t -> p c t"
                    ),
                )
            for ko in range(KO):
                xk = xt[:, ko // 2, ko % 2, :]
                st, sp_ = (ko == 0), (ko == KO - 1)
                nc.tensor.matmul(
                    q_ps0[:], wq_sb[:, ko, 0:P], xk, start=st, stop=sp_
                )
                nc.tensor.matmul(
                    q_ps1[:], wq_sb[:, ko, P:DQ], xk, start=st, stop=sp_
                )
                nc.tensor.matmul(
                    kv_ps[:], wkv_sb[:, ko, :], xk, start=st, stop=sp_
                )
            ts = slice(tb * TB, (tb + 1) * TB)
            nc.scalar.activation(qt_sb[:, 0, ts], q_ps0[:], AF.Copy, scale=SCALE)
            nc.scalar.activation(qt_sb[:, 1, ts], q_ps1[:], AF.Copy, scale=SCALE)
            nc.vector.tensor_copy(kkt_sb[0:HD, ts], kv_ps[0:HD, :])
            nc.vector.tensor_copy(vt_sb[HD:P, ts], kv_ps[HD:P, :])
            # duplicate KT to partitions 64..127 (SBUF->SBUF DMA moves partitions)
            nc.sync.dma_start(kkt_sb[HD:P, ts], kkt_sb[0:HD, ts])

        # ---------------- V transpose -> [kpos, hd|1] fp16 ----------------
        v1_sb = v_pool.tile([P, NKT, HD + 1], F16, tag="v1")
        nc.gpsimd.memset(v1_sb[:, :, HD : HD + 1], 1.0)
        for kt in range(NKT):
            tr_ps = wp.tile([P, TB], F32, tag="D")
            nc.tensor.transpose(
                tr_ps[:, 0:HD],
                vt_sb[HD:P, kt * P : (kt + 1) * P],
                ident[HD:P, HD:P],
            )
            nc.vector.tensor_copy(v1_sb[:, kt, 0:HD], tr_ps[:, 0:HD])

        # ---------------- attention + output proj, per q-block ----------------
        for qb in range(NT):
            qs = slice(qb * TB, (qb + 1) * TB)
            nkt = 4 * (qb + 1)  # causal: kpos tiles 0..nkt-1
            ot_sb = o_pool.tile([P, 2, TB], F16, tag="ot")
            for pair in range(2):
                o_ps = []
                for i in range(2):
                    o_ps_i = op.tile([P, TB], F32, tag="C", name=f"o_ps_{i}")
                    o_ps.append(o_ps_i)
                for kt in range(nkt):
                    ks = slice(kt * P, (kt + 1) * P)
                    s_ps0 = sp.tile([P, TB], F32, tag="A")
                    s_ps1 = sp.tile([P, TB], F32, tag="B")
                    nc.tensor.matmul(
                        s_ps0[:],
                        kkt_sb[0:HD, ks],
                        qt_sb[0:HD, pair, qs],
                        start=True,
                        stop=True,
                        tile_position=(0, 0),
                    )
                    nc.tensor.matmul(
                        s_ps1[:],
                        kkt_sb[HD:P, ks],
                        qt_sb[HD:P, pair, qs],
                        start=True,
                        stop=True,
                        tile_position=(64, 0),
                    )
                    pt0 = p_pool.tile([P, TB], F16, tag="p0")
                    pt1 = p_pool.tile([P, TB], F16, tag="p1")
                    nc.scalar.activation(pt0[:], s_ps0[:], AF.Exp)
                    nc.scalar.activation(pt1[:], s_ps1[:], AF.Exp)
                    if kt >= qb * 4:  # diagonal block: causal mask
                        for pt in (pt0, pt1):
                            nc.gpsimd.affine_select(
                                out=pt[:],
                                in_=pt[:],
                                compare_op=mybir.AluOpType.is_ge,
                                fill=0.0,
                                base=qb * TB - kt * P,
                                channel_multiplier=-1,
                                pattern=[[1, TB]],
                            )
                    st, sp_ = (kt == 0), (kt == nkt - 1)
                    nc.tensor.matmul(
                        o_ps[0][0 : HD + 1, :], v1_sb[:, kt, :], pt0[:],
                        start=st, stop=sp_,
                    )
                    nc.tensor.matmul(
                        o_ps[1][0 : HD + 1, :], v1_sb[:, kt, :], pt1[:],
                        start=st, stop=sp_,
                    )
                # normalize: ot[r] = o_ps[r][:64] / o_ps[r][64]
                for i in range(2):
                    sums = r_pool.tile([1, TB], F32, tag="sums")
                    nc.vector.tensor_copy(sums[:], o_ps[i][HD : HD + 1, :])
                    rb = r_pool.tile([HD, TB], F32, tag="rb")
                    nc.gpsimd.partition_broadcast(rb[:], sums[:])
                    nc.vector.reciprocal(rb[:], rb[:])
                    nc.vector.tensor_mul(
                        ot_sb[i * HD : (i + 1) * HD, pair, :],
                        o_ps[i][0:HD, :],
                        rb[:],
                    )
            # Wo partial for this q-block's 512 tokens
            for tt in range(4):
                rows = slice(qb * TB + tt * P, qb * TB + (tt + 1) * P)
                lslice = slice(tt * P, (tt + 1) * P)
                stg = p_pool.tile([P, 4, TB], F16, tag="stg")
                for nb in range(4):
                    wo_ps = wp.tile([P, TB], F32, tag="D")
                    for ko in range(2):
                        nc.tensor.matmul(
                            wo_ps[:],
                            ot_sb[:, ko, lslice],
                            wo_sb[:, ko, nb * TB : (nb + 1) * TB],
                            start=(ko == 0),
                            stop=(ko == 1),
                        )
                    nc.vector.tensor_copy(stg[:, nb, :], wo_ps[:])
                nc.sync.dma_start(partial[b, rows, :], stg[:])

    # ---------------- ReduceScatter the row-parallel Wo partials ------------
    nc.gpsimd.collective_compute(
        "ReduceScatter",
        mybir.AluOpType.add,
        replica_groups=[list(range(NCORES))],
        ins=[partial.opt()],
        outs=[rs_out.opt()],
    )
    nc.sync.dma_start(out, rs_out[:])


_NC_CACHE = {}


def get_nc():
    if "nc" not in _NC_CACHE:
        nc = bacc.Bacc("TRN2", target_bir_lowering=False, debug=False)
        with tile.TileContext(nc) as tc, ExitStack() as ctx:
            build_kernel(ctx, tc)
        nc.compile()
        _NC_CACHE["nc"] = nc
    return _NC_CACHE["nc"]


def _f16(a):
    a = np.asarray(a)
    return a if a.dtype == np.float16 else a.astype(np.float16)


def make_in_maps(x, Wq, Wk, Wv, Wo):
    # Convert each tensor to fp16 once (contiguous, vectorized); per-core
    # entries are views — run_bass_kernel_spmd's concat does the only copy.
    x, Wq, Wo = _f16(x), _f16(Wq), _f16(Wo)
    # [D, G, 2*HD] with K in cols :HD and V in HD:, so kv[:, g] is core g's wkv
    kv = np.concatenate(
        [_f16(Wk).reshape(D, G, HD), _f16(Wv).reshape(D, G, HD)], axis=2
    )
    in_maps = []
    for g in range(NCORES):
        dsl = slice(g * DQ, (g + 1) * DQ)
        in_maps.append(
            {"xs": x[:, :, dsl], "wq": Wq[:, dsl], "wkv": kv[:, g], "wo": Wo[dsl, :]}
        )
    return in_maps


def run(x, Wq, Wk, Wv, Wo, trace=False):
    nc = get_nc()
    in_maps = make_in_maps(x, Wq, Wk, Wv, Wo)
    res = run_bass_kernel_spmd(nc, in_maps, list(range(NCORES)), trace=trace)
    full = np.empty((B, T, D), np.float32)
    flat = full.reshape(B * T, D)
    for c, r in enumerate(res.results):
        flat[c * TS : (c + 1) * TS] = r["out"]
    return full, res


def kernel(x, Wq, Wk, Wv, Wo):
    return run(x, Wq, Wk, Wv, Wo)[0]
